# revision 37
# baseline (speedup 1.0000x reference)
"""CrossModalMatchingNetwork Trainium2 kernel.

Full-input contract: kernel(**inputs) takes the unsharded numpy inputs and
returns the full [B, S, S] cosine-similarity output (float32).

Strategy: data-parallel over batch across 8 NeuronCores (2 batches/core).
Host-side prep transposes the big activations to [D, S] layout so the
contraction dim lands on SBUF partitions, casts them to bf16 (fp32 PSUM
accumulation keeps the error ~5e-3), packs the projection weights into a
partition-major [P, K, H] layout (contiguous per partition, so weight DMAs
are ~128 fat descriptors instead of ~2000 1KB ones), and replicates the
weights to every core.

Per core, per batch (n2 indexes the two 512-column halves of S):
  projT: tT[h, s]  = sum_d WtT[d,h] * txtT[d,s] + bt[h]   (per-half tiles)
  T-norm chain per half (vector): tsq -> tss -> ones-matmul (partition sum)
     -> sqrt row -> replicate via K=1 matmul -> reciprocal -> fold 1/tn
     into tT before dots.
  projV + V-norm chain: squares/sums -> ones-matmul -> sqrt -> reciprocal
     ROW (f32, then a bf16 copy) -> PE K=1 transposes turn the row into
     per-i-block [P,1] columns; 1/vn is applied as the activation `scale`
     operand in the dots epilogue, so NO fold of vT gates the dots matmuls.
  dots i-loop: psum = vT_i^T @ tT -> scale-copy to SBUF -> DMA out
     (out DMAs alternate gpsimd/sync queues; last tile split per half).

Batch 0 is latency-critical, so its inputs stream in per-k chunks in
priority order (txt+wt first, then wv+vis) and the first projection halves
run k-OUTER across four concurrent PSUM accumulation groups, consuming
each chunk as it lands — the PE starts ~8.5us in instead of ~18us.
The kernel() entry point executes the NEFF twice and only accepts a
result confirmed by a second run (first-exec-after-compile occasionally
returns a partially corrupted tile).
"""

import numpy as np
from contextlib import ExitStack

import concourse.bass as bass
import concourse.mybir as mybir
import concourse.tile as tile
from concourse import bacc
from concourse.bass import ds, ts

B, S, VD, TD, H = 16, 1024, 1024, 768, 512
NCORES = 8
BPC = B // NCORES  # batches per core
P = 128
FD = 512  # matmul moving-operand free dim (one PSUM bank of fp32)

F32 = mybir.dt.float32
F32R = mybir.dt.float32r
BF16 = mybir.dt.bfloat16

AF = mybir.ActivationFunctionType

N_WARMUP = 14


def build(bpc=BPC, s=S, vd=VD, td=TD, h=H, dtype="bf16"):
    fd = min(FD, s)
    kv, kt, mh = vd // P, td // P, h // P
    ns, ms = s // fd, s // P
    ipn = fd // P  # i-blocks per S half

    if dtype == "bf16":
        CT = BF16
        _w = lambda ap: ap  # noqa: E731
    else:
        CT = F32
        _w = lambda ap: ap.bitcast(F32R)  # noqa: E731

    nc = bacc.Bacc("TRN2", target_bir_lowering=False)
    visT = nc.dram_tensor("visT", [bpc, vd, s], CT, kind="ExternalInput")
    txtT = nc.dram_tensor("txtT", [bpc, td, s], CT, kind="ExternalInput")
    wvp = nc.dram_tensor("wvp", [P, kv, h], CT, kind="ExternalInput")
    wtp = nc.dram_tensor("wtp", [P, kt, h], CT, kind="ExternalInput")
    bvp = nc.dram_tensor("bvp", [P, mh], F32, kind="ExternalInput")
    btp = nc.dram_tensor("btp", [P, mh], F32, kind="ExternalInput")
    onesd = nc.dram_tensor("ones", [P, P], CT, kind="ExternalInput")
    out = nc.dram_tensor("out", [bpc, s, s], F32, kind="ExternalOutput")

    with (
        tile.TileContext(nc) as tc,
        ExitStack() as ctx,
        nc.allow_low_precision(reason="compute dtype is bf16 by design"),
    ):
        consts = ctx.enter_context(tc.tile_pool(name="consts", bufs=1))
        xin_pool = ctx.enter_context(tc.tile_pool(name="xin", bufs=2))
        proj_pool = ctx.enter_context(tc.tile_pool(name="proj", bufs=1))
        work_pool = ctx.enter_context(tc.tile_pool(name="work", bufs=2))
        row_pool = work_pool
        out_pool = ctx.enter_context(tc.tile_pool(name="outs", bufs=4))
        ps_mm = ctx.enter_context(tc.tile_pool(name="ps_mm", bufs=6, space="PSUM"))
        ps_row = ctx.enter_context(tc.tile_pool(name="ps_row", bufs=1, space="PSUM"))
        ps_repl = ctx.enter_context(tc.tile_pool(name="ps_repl", bufs=1, space="PSUM"))

        # --- consts: tiny ones on the gpsimd queue, wt per-k chunks on scalar
        bt_sb = consts.tile([P, mh], F32)
        nc.gpsimd.dma_start(bt_sb[:], btp[:, :])
        bv_sb = consts.tile([P, mh], F32)
        nc.gpsimd.dma_start(bv_sb[:], bvp[:, :])
        ones_sb = consts.tile([P, P], CT)
        nc.gpsimd.dma_start(_w(ones_sb[:]), _w(onesd[:, :]))
        wt_sb = consts.tile([P, kt, h], CT)
        for k in range(kt):
            nc.scalar.dma_start(_w(wt_sb[:, k, :]), _w(wtp[:, k, :]))
        wv_sb = consts.tile([P, kv, h], CT)
        ones_col = ones_sb[:, 0:1]
        ones_row = ones_sb[0:1, :]

        # txt b0 chunks first: projT is the critical path at the head
        txt0_sb = xin_pool.tile([P, kt, s], CT, tag="txt")
        for k in range(kt):
            nc.sync.dma_start(_w(txt0_sb[:, k, :]), _w(txtT[0, ds(k * P, P), :]))

        # one_ct: moving operand for the K=1 row->column transposes
        one_ct = consts.tile([1, 1], CT)
        nc.vector.memset(one_ct[:], 1.0)

        # PE warm-up while the first chunks land (clock ramp)
        warm_sb = consts.tile([P, fd], CT)
        nc.vector.memset(warm_sb[:], 0.0)
        warm_ps = ps_repl.tile([P, fd], F32, tag="ps_repl")
        for _ in range(N_WARMUP):
            nc.tensor.matmul(warm_ps[:], _w(warm_sb[:, 0:P]), _w(warm_sb[:]))
        nc.scalar.activation(_w(warm_sb[:, 0:P]), warm_ps[:, 0:P], AF.Copy)
        # prime the Abs_reciprocal_sqrt activation table now (it loads
        # lazily at first use, which otherwise stalls the first norm chain
        # ~1.3us mid-kernel)
        warm_r = consts.tile([1, 1], CT)
        nc.scalar.activation(
            _w(warm_r[:]), warm_ps[0:1, 0:1], AF.Abs_reciprocal_sqrt
        )

        def proj_group(n2, m, kk, w_sb, b_sb, x_sb, y_half):
            """y_half[:, m, :] = W[:, :, m-slice].T @ x[n2] + b (one group)."""
            pv = ps_mm.tile([P, fd], F32, tag="ps_mm")
            for k in range(kk):
                nc.tensor.matmul(
                    pv[:],
                    _w(w_sb[:, k, ts(m, P)]),
                    _w(x_sb[:, k, ds(n2 * fd, fd)]),
                    start=(k == 0),
                    stop=(k == kk - 1),
                )
            nc.scalar.activation(
                _w(y_half[:, m, :]), pv[:], AF.Identity,
                bias=b_sb[:, ds(m, 1)],
            )

        def proj_kouter(n2, kk, w_sb, b_sb, x_sb, y_half):
            """All mh groups at once, k outermost for the first half of k so
            each input chunk is consumed as its DMA lands (batch-0 head
            latency); per-m k-inner tail so the drain activations interleave
            instead of bursting at the end."""
            pvs = [
                ps_mm.tile([P, fd], F32, tag="ps_mm", name=f"pko{m}")
                for m in range(mh)
            ]
            for k in range(kk):
                for m in range(mh):
                    nc.tensor.matmul(
                        pvs[m][:],
                        _w(w_sb[:, k, ts(m, P)]),
                        _w(x_sb[:, k, ds(n2 * fd, fd)]),
                        start=(k == 0),
                        stop=(k == kk - 1),
                    )
            for m in range(mh):
                nc.scalar.activation(
                    _w(y_half[:, m, :]), pvs[m][:], AF.Identity,
                    bias=b_sb[:, ds(m, 1)],
                )

        # ---- T-side norm chain (fold 1/tn into tT before dots), on vector
        def t_pre(y_half, n2):
            sq = work_pool.tile([P, mh, fd], CT, tag=f"sqt{n2}")
            nc.vector.tensor_mul(_w(sq[:]), y_half[:], y_half[:])
            ss = work_pool.tile([P, fd], CT, tag=f"sst{n2}")
            nc.vector.tensor_add(_w(ss[:]), sq[:, 0, :], sq[:, 1, :])
            for m in range(2, mh):
                nc.vector.tensor_add(_w(ss[:]), ss[:], sq[:, m, :])
            return ss

        def t_mid(ss, n2):
            pn = ps_row.tile([1, fd], F32, tag="ps_row")
            nc.tensor.matmul(pn[:], _w(ones_col), _w(ss[:]))
            # 1/sqrt in one activation (Abs_reciprocal_sqrt); the replicate
            # matmul then broadcasts 1/tn directly, no DVE reciprocal needed
            nrow = row_pool.tile([1, fd], CT, tag=f"nt{n2}")
            nc.scalar.activation(_w(nrow[:]), pn[:], AF.Abs_reciprocal_sqrt)
            rp = ps_repl.tile([P, fd], F32, tag="ps_repl")
            nc.tensor.matmul(rp[:], _w(ones_row), _w(nrow[:]))
            return rp

        def t_fin(y_half, rp, n2):
            for m in range(mh):
                nc.vector.tensor_mul(
                    _w(y_half[:, m, :]), y_half[:, m, :], rp[:]
                )

        # ---- V-side norm chain: squares/sums on gpsimd, reciprocal row on
        # vector, then PE K=1 transposes produce per-i-block [P,1] columns of
        # 1/vn consumed as the epilogue activation scale.
        def v_pre(y_half, n2):
            sq = work_pool.tile([P, mh, fd], CT, tag=f"sqv{n2}")
            nc.vector.tensor_mul(_w(sq[:]), y_half[:], y_half[:])
            ss = work_pool.tile([P, fd], CT, tag=f"ssv{n2}")
            nc.vector.tensor_add(_w(ss[:]), sq[:, 0, :], sq[:, 1, :])
            for m in range(2, mh):
                nc.vector.tensor_add(_w(ss[:]), ss[:], sq[:, m, :])
            return ss

        def v_ones(ss):
            pn = ps_row.tile([1, fd], F32, tag="ps_row")
            nc.tensor.matmul(pn[:], _w(ones_col), _w(ss[:]))
            return pn

        def v_recip(pn, n2):
            rrow_ct = row_pool.tile([1, fd], CT, tag=f"rvc{n2}")
            nc.scalar.activation(_w(rrow_ct[:]), pn[:], AF.Abs_reciprocal_sqrt)
            return rrow_ct

        def v_cols(rrow, rvn_cols, n2):
            pt = ps_row.tile([P, ipn], F32, tag="ps_row")
            for c in range(ipn):
                nc.tensor.matmul(
                    pt[:, ds(c, 1)],
                    _w(rrow[0:1, ds(c * P, P)]),
                    _w(one_ct[0:1, 0:1]),
                )
            nc.scalar.activation(
                rvn_cols[:, ds(n2 * ipn, ipn)], pt[:], AF.Copy
            )

        # ---- dots
        def dots_groups(i, jc, vts, tts):
            vt_h = vts[i // ipn]
            pd = ps_mm.tile([P, fd], F32, tag="ps_mm")
            for hc in range(mh):
                nc.tensor.matmul(
                    pd[:],
                    _w(vt_h[:, hc, ts(i % ipn, P)]),
                    _w(tts[jc][:, hc, :]),
                    start=(hc == 0),
                    stop=(hc == mh - 1),
                )
            return pd

        def dots_pair(i, vts, tts):
            """Both jc groups hc-outer: one stationary load serves two
            512-col streams into alternating PSUM banks, hiding each bank's
            accumulation-group turnaround behind the other's stream."""
            vt_h = vts[i // ipn]
            pd0 = ps_mm.tile([P, fd], F32, tag="ps_mm", name=f"pda{i}")
            pd1 = ps_mm.tile([P, fd], F32, tag="ps_mm", name=f"pdb{i}")
            for hc in range(mh):
                for jc, pd in ((0, pd0), (1, pd1)):
                    nc.tensor.matmul(
                        pd[:],
                        _w(vt_h[:, hc, ts(i % ipn, P)]),
                        _w(tts[jc][:, hc, :]),
                        start=(hc == 0),
                        stop=(hc == mh - 1),
                    )
            return [pd0, pd1]

        def dots_epi(b, i, pds, rvn_cols, last=False):
            col = rvn_cols[:, ds(i, 1)]
            out_sb = out_pool.tile([P, s], F32)
            dq = nc.gpsimd if i % 2 == 0 else nc.sync
            if last:
                # final tiles: half-granularity DMAs spread over three queues
                # (scalar is idle at the end) so no single queue holds a
                # multi-tile backlog after the last matmul
                q0, q1 = {
                    ms - 4: (nc.gpsimd, nc.scalar),
                    ms - 3: (nc.sync, nc.scalar),
                    ms - 2: (nc.gpsimd, nc.sync),
                    ms - 1: (nc.gpsimd, nc.sync),
                }.get(i, (nc.gpsimd, nc.sync))
                nc.scalar.activation(out_sb[:, ds(0, fd)], pds[0][:], AF.Copy,
                                     scale=col)
                q0.dma_start(out[b, ts(i, P), ds(0, fd)],
                             out_sb[:, ds(0, fd)])
                nc.vector.tensor_scalar_mul(out_sb[:, ds(fd, fd)], pds[1][:],
                                            col)
                q1.dma_start(out[b, ts(i, P), ds(fd, fd)],
                             out_sb[:, ds(fd, fd)])
                return
            nc.scalar.activation(out_sb[:, ds(0, fd)], pds[0][:], AF.Copy,
                                 scale=col)
            if i < ms // 2:
                nc.scalar.activation(out_sb[:, ds(fd, fd)], pds[1][:], AF.Copy,
                                     scale=col)
            else:
                nc.vector.tensor_scalar_mul(out_sb[:, ds(fd, fd)], pds[1][:],
                                            col)
            dq.dma_start(out[b, ts(i, P), :], out_sb[:])

        def body(b, txt_sb, vis_sb, kouter, emit_dmas=None):
            vt0 = proj_pool.tile([P, mh, fd], CT, tag="vt0")
            vt1 = proj_pool.tile([P, mh, fd], CT, tag="vt1")
            tt0 = proj_pool.tile([P, mh, fd], CT, tag="tt0")
            tt1 = proj_pool.tile([P, mh, fd], CT, tag="tt1")
            rvn_cols = row_pool.tile([P, ms], F32, tag="rvncols")
            vts, tts = (vt0, vt1), (tt0, tt1)

            if kouter:
                proj_kouter(0, kt, wt_sb, bt_sb, txt_sb, tt0)
            else:
                for m in range(mh):
                    proj_group(0, m, kt, wt_sb, bt_sb, txt_sb, tt0)
            proj_group(1, 0, kt, wt_sb, bt_sb, txt_sb, tt1)
            ss_t0 = t_pre(tt0, 0)
            proj_group(1, 1, kt, wt_sb, bt_sb, txt_sb, tt1)
            rp_t0 = t_mid(ss_t0, 0)
            proj_group(1, 2, kt, wt_sb, bt_sb, txt_sb, tt1)
            t_fin(tt0, rp_t0, 0)
            proj_group(1, 3, kt, wt_sb, bt_sb, txt_sb, tt1)
            if emit_dmas is not None:
                emit_dmas()

            if kouter:
                proj_kouter(0, kv, wv_sb, bv_sb, vis_sb, vt0)
            else:
                for m in range(mh):
                    proj_group(0, m, kv, wv_sb, bv_sb, vis_sb, vt0)
            ss_t1 = t_pre(tt1, 1)
            ss_v0 = v_pre(vt0, 0)
            proj_group(1, 0, kv, wv_sb, bv_sb, vis_sb, vt1)
            proj_group(1, 1, kv, wv_sb, bv_sb, vis_sb, vt1)
            rp_t1 = t_mid(ss_t1, 1)
            proj_group(1, 2, kv, wv_sb, bv_sb, vis_sb, vt1)
            t_fin(tt1, rp_t1, 1)
            proj_group(1, 3, kv, wv_sb, bv_sb, vis_sb, vt1)

            # dots: first two i-blocks interleaved [i0jc0, i1jc0, i0jc1,
            # i1jc1] so the tt1-fold deadline lands two groups later; the V0
            # column chain threads through the slots between groups.
            pd00 = dots_groups(0, 0, vts, tts)
            pn_v0 = v_ones(ss_v0)
            pd10 = dots_groups(1, 0, vts, tts)
            rrow_v0 = v_recip(pn_v0, 0)
            pd01 = dots_groups(0, 1, vts, tts)
            v_cols(rrow_v0, rvn_cols, 0)
            pd11 = dots_groups(1, 1, vts, tts)
            ss_v1 = v_pre(vt1, 1)
            dots_epi(b, 0, [pd00, pd01], rvn_cols)
            dots_epi(b, 1, [pd10, pd11], rvn_cols)

            pds = dots_pair(2, vts, tts)
            pn_v1 = v_ones(ss_v1)
            dots_epi(b, 2, pds, rvn_cols)
            pds = dots_pair(3, vts, tts)
            rrow_v1 = v_recip(pn_v1, 1)
            dots_epi(b, 3, pds, rvn_cols)
            pds = dots_pair(4, vts, tts)
            v_cols(rrow_v1, rvn_cols, 1)
            dots_epi(b, 4, pds, rvn_cols, last=(b == bpc - 1 and 4 >= ms - 4))
            for i in range(5, ms):
                pds = dots_pair(i, vts, tts)
                dots_epi(b, i, pds, rvn_cols,
                         last=(b == bpc - 1 and i >= ms - 4))

        # ---------------- batch 0: chunk-paced head ----------------
        vis0_sb = xin_pool.tile([P, kv, s], CT, tag="vis")
        txt1_sb = xin_pool.tile([P, kt, s], CT, tag="txt")
        vis1_sb = xin_pool.tile([P, kv, s], CT, tag="vis")

        def emit_b0_dmas():
            # wv then vis b0 (two halves: the k-outer pass consumes at half
            # granularity); coarse DMAs so doorbell issue ops (~0.7us each)
            # don't pace the feed. Whole-tensor b1 loads queue up behind.
            nc.sync.dma_start(_w(wv_sb[:]), _w(wvp[:, :, :]))
            half = kv // 2
            nc.sync.dma_start(
                _w(vis0_sb[:, ds(0, half), :]),
                _w(visT[0, ds(0, half * P), :].rearrange("(k p) s -> p k s", p=P)),
            )
            nc.sync.dma_start(
                _w(vis0_sb[:, ds(half, half), :]),
                _w(visT[0, ds(half * P, half * P), :].rearrange(
                    "(k p) s -> p k s", p=P)),
            )
            nc.sync.dma_start(
                _w(txt1_sb[:]), _w(txtT[1, :, :].rearrange("(k p) s -> p k s", p=P))
            )
            nc.sync.dma_start(
                _w(vis1_sb[:]), _w(visT[1, :, :].rearrange("(k p) s -> p k s", p=P))
            )

        body(0, txt0_sb, vis0_sb, kouter=True, emit_dmas=emit_b0_dmas)
        body(1, txt1_sb, vis1_sb, kouter=False)

    nc.compile()
    return nc


_CACHE = {}


def _get_nc(dtype="bf16"):
    if dtype not in _CACHE:
        _CACHE[dtype] = build(dtype=dtype)
    return _CACHE[dtype]


def _prep_in_maps(visual_features, text_features, Wv, bv, Wt, bt, dtype="bf16"):
    import ml_dtypes

    f = np.float32
    ct = ml_dtypes.bfloat16 if dtype == "bf16" else f
    kv, kt, mh = VD // P, TD // P, H // P
    wvp = np.ascontiguousarray(
        np.asarray(Wv, dtype=f).T.reshape(kv, P, H).transpose(1, 0, 2)
    ).astype(ct)
    wtp = np.ascontiguousarray(
        np.asarray(Wt, dtype=f).T.reshape(kt, P, H).transpose(1, 0, 2)
    ).astype(ct)
    bvp = np.ascontiguousarray(np.asarray(bv, dtype=f).reshape(mh, P).T)
    btp = np.ascontiguousarray(np.asarray(bt, dtype=f).reshape(mh, P).T)
    ones = np.ones((P, P), dtype=np.float32).astype(ct)
    vis = np.asarray(visual_features, dtype=f)
    txt = np.asarray(text_features, dtype=f)
    in_maps = []
    for c in range(NCORES):
        sl = slice(c * BPC, (c + 1) * BPC)
        in_maps.append({
            "visT": np.ascontiguousarray(vis[sl].transpose(0, 2, 1)).astype(ct),
            "txtT": np.ascontiguousarray(txt[sl].transpose(0, 2, 1)).astype(ct),
            "wvp": wvp,
            "wtp": wtp,
            "bvp": bvp,
            "btp": btp,
            "ones": ones,
        })
    return in_maps


def _exec(nc, in_maps, trace, tmpdir):
    from concourse.bass_utils import run_bass_kernel_spmd

    res = run_bass_kernel_spmd(
        nc, in_maps, core_ids=list(range(NCORES)), trace=trace, tmpdir=tmpdir
    )
    outp = np.concatenate([res.results[c]["out"] for c in range(NCORES)], axis=0)
    return outp, res


def run(inputs, trace=False, tmpdir=None, dtype="bf16"):
    """Returns (full_output, BassKernelResults)."""
    nc = _get_nc(dtype)
    in_maps = _prep_in_maps(**inputs, dtype=dtype)
    return _exec(nc, in_maps, trace, tmpdir)


def kernel(**inputs) -> np.ndarray:
    """Full-input entry point. The first execution of a freshly-compiled
    NEFF occasionally returns a partially corrupted tile (observed ~2x in
    ~30 runs, always on the first exec), so execute twice and only accept
    a result confirmed by a second run."""
    nc = _get_nc("bf16")
    in_maps = _prep_in_maps(**inputs, dtype="bf16")
    prev = None
    for attempt in range(4):
        outp, _ = _exec(nc, in_maps, trace=False, tmpdir=None)
        if prev is not None:
            denom = np.linalg.norm(prev)
            if denom == 0.0 or np.linalg.norm(outp - prev) / denom < 1e-6:
                return outp
        prev = outp
    return prev


# revision 38
# speedup vs baseline: 1.1647x; 1.1647x over previous
"""CrossModalMatchingNetwork Trainium2 kernel.

Full-input contract: kernel(**inputs) takes the unsharded numpy inputs and
returns the full [B, S, S] cosine-similarity output (float32).

Strategy: data-parallel over batch across 8 NeuronCores (2 batches/core).
Host-side prep transposes the big activations to [D, S] layout so the
contraction dim lands on SBUF partitions, casts them to bf16 (fp32 PSUM
accumulation keeps the error ~5e-3), packs the projection weights into a
partition-major [P, K, H] layout (contiguous per partition, so weight DMAs
are ~128 fat descriptors instead of ~2000 1KB ones), and replicates the
weights to every core.

Per core, per batch (n2 indexes the two 512-column halves of S):
  projT: tT[h, s]  = sum_d WtT[d,h] * txtT[d,s] + bt[h]   (per-half tiles)
  T-norm chain per half (vector): tsq -> tss -> ones-matmul (partition sum)
     -> sqrt row -> replicate via K=1 matmul -> reciprocal -> fold 1/tn
     into tT before dots.
  projV + V-norm chain: squares/sums -> ones-matmul -> sqrt -> reciprocal
     ROW (f32, then a bf16 copy) -> PE K=1 transposes turn the row into
     per-i-block [P,1] columns; 1/vn is applied as the activation `scale`
     operand in the dots epilogue, so NO fold of vT gates the dots matmuls.
  dots i-loop: psum = vT_i^T @ tT -> scale-copy to SBUF -> DMA out
     (out DMAs alternate gpsimd/sync queues; last tile split per half).

Batch 0 is latency-critical, so its inputs stream in per-k chunks in
priority order (txt+wt first, then wv+vis) and the first projection halves
run k-OUTER across four concurrent PSUM accumulation groups, consuming
each chunk as it lands — the PE starts ~8.5us in instead of ~18us.
The kernel() entry point executes the NEFF twice and only accepts a
result confirmed by a second run (first-exec-after-compile occasionally
returns a partially corrupted tile).
"""

import numpy as np
from contextlib import ExitStack

import concourse.bass as bass
import concourse.mybir as mybir
import concourse.tile as tile
from concourse import bacc
from concourse.bass import ds, ts

B, S, VD, TD, H = 16, 1024, 1024, 768, 512
NCORES = 8
BPC = B // NCORES  # batches per core
P = 128
FD = 512  # matmul moving-operand free dim (one PSUM bank of fp32)

F32 = mybir.dt.float32
F32R = mybir.dt.float32r
BF16 = mybir.dt.bfloat16

AF = mybir.ActivationFunctionType

N_WARMUP = 14


def build(bpc=BPC, s=S, vd=VD, td=TD, h=H, dtype="bf16"):
    fd = min(FD, s)
    kv, kt, mh = vd // P, td // P, h // P
    ns, ms = s // fd, s // P
    ipn = fd // P  # i-blocks per S half

    if dtype == "bf16":
        CT = BF16
        _w = lambda ap: ap  # noqa: E731
    else:
        CT = F32
        _w = lambda ap: ap.bitcast(F32R)  # noqa: E731

    nc = bacc.Bacc("TRN2", target_bir_lowering=False)
    visT = nc.dram_tensor("visT", [bpc, vd, s], CT, kind="ExternalInput")
    txtT = nc.dram_tensor("txtT", [bpc, td, s], CT, kind="ExternalInput")
    wvp = nc.dram_tensor("wvp", [P, kv, h], CT, kind="ExternalInput")
    wtp = nc.dram_tensor("wtp", [P, kt, h], CT, kind="ExternalInput")
    bvp = nc.dram_tensor("bvp", [P, mh], F32, kind="ExternalInput")
    btp = nc.dram_tensor("btp", [P, mh], F32, kind="ExternalInput")
    onesd = nc.dram_tensor("ones", [P, P], CT, kind="ExternalInput")
    out = nc.dram_tensor("out", [bpc, s, s], F32, kind="ExternalOutput")

    with (
        tile.TileContext(nc) as tc,
        ExitStack() as ctx,
        nc.allow_low_precision(reason="compute dtype is bf16 by design"),
    ):
        consts = ctx.enter_context(tc.tile_pool(name="consts", bufs=1))
        xin_pool = ctx.enter_context(tc.tile_pool(name="xin", bufs=2))
        proj_pool = ctx.enter_context(tc.tile_pool(name="proj", bufs=1))
        work_pool = ctx.enter_context(tc.tile_pool(name="work", bufs=2))
        row_pool = work_pool
        out_pool = ctx.enter_context(tc.tile_pool(name="outs", bufs=4))
        ps_mm = ctx.enter_context(tc.tile_pool(name="ps_mm", bufs=6, space="PSUM"))
        ps_row = ctx.enter_context(tc.tile_pool(name="ps_row", bufs=1, space="PSUM"))
        ps_repl = ctx.enter_context(tc.tile_pool(name="ps_repl", bufs=1, space="PSUM"))

        # --- consts: tiny ones on the gpsimd queue, wt per-k chunks on scalar
        bt_sb = consts.tile([P, mh], F32)
        nc.gpsimd.dma_start(bt_sb[:], btp[:, :])
        bv_sb = consts.tile([P, mh], F32)
        nc.gpsimd.dma_start(bv_sb[:], bvp[:, :])
        ones_sb = consts.tile([P, P], CT)
        nc.gpsimd.dma_start(_w(ones_sb[:]), _w(onesd[:, :]))
        wt_sb = consts.tile([P, kt, h], CT)
        for k in range(kt):
            nc.scalar.dma_start(_w(wt_sb[:, k, :]), _w(wtp[:, k, :]))
        wv_sb = consts.tile([P, kv, h], CT)
        ones_col = ones_sb[:, 0:1]
        ones_row = ones_sb[0:1, :]

        # txt b0 chunks first: projT is the critical path at the head
        txt0_sb = xin_pool.tile([P, kt, s], CT, tag="txt")
        for k in range(kt):
            nc.sync.dma_start(_w(txt0_sb[:, k, :]), _w(txtT[0, ds(k * P, P), :]))

        # one_ct: moving operand for the K=1 row->column transposes
        one_ct = consts.tile([1, 1], CT)
        nc.vector.memset(one_ct[:], 1.0)

        # PE warm-up while the first chunks land (clock ramp)
        warm_sb = consts.tile([P, fd], CT)
        nc.vector.memset(warm_sb[:], 0.0)
        warm_ps = ps_repl.tile([P, fd], F32, tag="ps_repl")
        for _ in range(N_WARMUP):
            nc.tensor.matmul(warm_ps[:], _w(warm_sb[:, 0:P]), _w(warm_sb[:]))
        nc.scalar.activation(_w(warm_sb[:, 0:P]), warm_ps[:, 0:P], AF.Copy)
        # prime the Abs_reciprocal_sqrt activation table now (it loads
        # lazily at first use, which otherwise stalls the first norm chain
        # ~1.3us mid-kernel)
        warm_r = consts.tile([1, 1], CT)
        nc.scalar.activation(
            _w(warm_r[:]), warm_ps[0:1, 0:1], AF.Abs_reciprocal_sqrt
        )

        def proj_group(n2, m, kk, w_sb, b_sb, x_sb, y_half):
            """y_half[:, m, :] = W[:, :, m-slice].T @ x[n2] + b (one group)."""
            pv = ps_mm.tile([P, fd], F32, tag="ps_mm")
            for k in range(kk):
                nc.tensor.matmul(
                    pv[:],
                    _w(w_sb[:, k, ts(m, P)]),
                    _w(x_sb[:, k, ds(n2 * fd, fd)]),
                    start=(k == 0),
                    stop=(k == kk - 1),
                )
            nc.scalar.activation(
                _w(y_half[:, m, :]), pv[:], AF.Identity,
                bias=b_sb[:, ds(m, 1)],
            )

        def proj_kouter(n2, kk, w_sb, b_sb, x_sb, y_half):
            """All mh groups at once, k outermost for the first half of k so
            each input chunk is consumed as its DMA lands (batch-0 head
            latency); per-m k-inner tail so the drain activations interleave
            instead of bursting at the end."""
            pvs = [
                ps_mm.tile([P, fd], F32, tag="ps_mm", name=f"pko{m}")
                for m in range(mh)
            ]
            for k in range(kk):
                for m in range(mh):
                    nc.tensor.matmul(
                        pvs[m][:],
                        _w(w_sb[:, k, ts(m, P)]),
                        _w(x_sb[:, k, ds(n2 * fd, fd)]),
                        start=(k == 0),
                        stop=(k == kk - 1),
                    )
            for m in range(mh):
                nc.scalar.activation(
                    _w(y_half[:, m, :]), pvs[m][:], AF.Identity,
                    bias=b_sb[:, ds(m, 1)],
                )

        # ---- T-side norm chain (fold 1/tn into tT before dots), on vector
        def t_pre(y_half, n2):
            sq = work_pool.tile([P, mh, fd], CT, tag=f"sqt{n2}")
            nc.vector.tensor_mul(_w(sq[:]), y_half[:], y_half[:])
            ss = work_pool.tile([P, fd], CT, tag=f"sst{n2}")
            nc.vector.tensor_add(_w(ss[:]), sq[:, 0, :], sq[:, 1, :])
            for m in range(2, mh):
                nc.vector.tensor_add(_w(ss[:]), ss[:], sq[:, m, :])
            return ss

        def t_mid(ss, n2):
            pn = ps_row.tile([1, fd], F32, tag="ps_row")
            nc.tensor.matmul(pn[:], _w(ones_col), _w(ss[:]))
            # 1/sqrt in one activation (Abs_reciprocal_sqrt); the replicate
            # matmul then broadcasts 1/tn directly, no DVE reciprocal needed
            nrow = row_pool.tile([1, fd], CT, tag=f"nt{n2}")
            nc.scalar.activation(_w(nrow[:]), pn[:], AF.Abs_reciprocal_sqrt)
            rp = ps_repl.tile([P, fd], F32, tag="ps_repl")
            nc.tensor.matmul(rp[:], _w(ones_row), _w(nrow[:]))
            return rp

        def t_fin(y_half, rp, n2):
            for m in range(mh):
                nc.vector.tensor_mul(
                    _w(y_half[:, m, :]), y_half[:, m, :], rp[:]
                )

        # ---- V-side norm chain: squares/sums on gpsimd, reciprocal row on
        # vector, then PE K=1 transposes produce per-i-block [P,1] columns of
        # 1/vn consumed as the epilogue activation scale.
        def v_pre(y_half, n2):
            sq = work_pool.tile([P, mh, fd], CT, tag=f"sqv{n2}")
            nc.vector.tensor_mul(_w(sq[:]), y_half[:], y_half[:])
            ss = work_pool.tile([P, fd], CT, tag=f"ssv{n2}")
            nc.vector.tensor_add(_w(ss[:]), sq[:, 0, :], sq[:, 1, :])
            for m in range(2, mh):
                nc.vector.tensor_add(_w(ss[:]), ss[:], sq[:, m, :])
            return ss

        def v_ones(ss):
            pn = ps_row.tile([1, fd], F32, tag="ps_row")
            nc.tensor.matmul(pn[:], _w(ones_col), _w(ss[:]))
            return pn

        def v_recip(pn, n2):
            rrow_ct = row_pool.tile([1, fd], CT, tag=f"rvc{n2}")
            nc.scalar.activation(_w(rrow_ct[:]), pn[:], AF.Abs_reciprocal_sqrt)
            return rrow_ct

        def v_cols(rrow, rvn_cols, n2):
            pt = ps_row.tile([P, ipn], F32, tag="ps_row")
            for c in range(ipn):
                nc.tensor.matmul(
                    pt[:, ds(c, 1)],
                    _w(rrow[0:1, ds(c * P, P)]),
                    _w(one_ct[0:1, 0:1]),
                )
            nc.scalar.activation(
                rvn_cols[:, ds(n2 * ipn, ipn)], pt[:], AF.Copy
            )

        # ---- dots
        def dots_groups(i, jc, vts, tts):
            vt_h = vts[i // ipn]
            pd = ps_mm.tile([P, fd], F32, tag="ps_mm")
            for hc in range(mh):
                nc.tensor.matmul(
                    pd[:],
                    _w(vt_h[:, hc, ts(i % ipn, P)]),
                    _w(tts[jc][:, hc, :]),
                    start=(hc == 0),
                    stop=(hc == mh - 1),
                )
            return pd

        def dots_pair(i, vts, tts):
            """Both jc groups hc-outer: one stationary load serves two
            512-col streams into alternating PSUM banks, hiding each bank's
            accumulation-group turnaround behind the other's stream."""
            vt_h = vts[i // ipn]
            pd0 = ps_mm.tile([P, fd], F32, tag="ps_mm", name=f"pda{i}")
            pd1 = ps_mm.tile([P, fd], F32, tag="ps_mm", name=f"pdb{i}")
            for hc in range(mh):
                for jc, pd in ((0, pd0), (1, pd1)):
                    nc.tensor.matmul(
                        pd[:],
                        _w(vt_h[:, hc, ts(i % ipn, P)]),
                        _w(tts[jc][:, hc, :]),
                        start=(hc == 0),
                        stop=(hc == mh - 1),
                    )
            return [pd0, pd1]

        def dots_epi(b, i, pds, rvn_cols, last=False):
            col = rvn_cols[:, ds(i, 1)]
            out_sb = out_pool.tile([P, s], F32)
            dq = nc.gpsimd if i % 2 == 0 else nc.sync
            if last:
                # final tiles: half-granularity DMAs spread over three queues
                # (scalar is idle at the end) so no single queue holds a
                # multi-tile backlog after the last matmul
                q0, q1 = {
                    ms - 4: (nc.gpsimd, nc.scalar),
                    ms - 3: (nc.sync, nc.scalar),
                    ms - 2: (nc.gpsimd, nc.sync),
                    ms - 1: (nc.gpsimd, nc.sync),
                }.get(i, (nc.gpsimd, nc.sync))
                nc.scalar.activation(out_sb[:, ds(0, fd)], pds[0][:], AF.Copy,
                                     scale=col)
                q0.dma_start(out[b, ts(i, P), ds(0, fd)],
                             out_sb[:, ds(0, fd)])
                nc.vector.tensor_scalar_mul(out_sb[:, ds(fd, fd)], pds[1][:],
                                            col)
                q1.dma_start(out[b, ts(i, P), ds(fd, fd)],
                             out_sb[:, ds(fd, fd)])
                return
            nc.scalar.activation(out_sb[:, ds(0, fd)], pds[0][:], AF.Copy,
                                 scale=col)
            if i < ms // 2:
                nc.scalar.activation(out_sb[:, ds(fd, fd)], pds[1][:], AF.Copy,
                                     scale=col)
            else:
                nc.vector.tensor_scalar_mul(out_sb[:, ds(fd, fd)], pds[1][:],
                                            col)
            dq.dma_start(out[b, ts(i, P), :], out_sb[:])

        def body(b, txt_sb, vis_sb, kouter, emit_dmas=None):
            vt0 = proj_pool.tile([P, mh, fd], CT, tag="vt0")
            vt1 = proj_pool.tile([P, mh, fd], CT, tag="vt1")
            tt0 = proj_pool.tile([P, mh, fd], CT, tag="tt0")
            tt1 = proj_pool.tile([P, mh, fd], CT, tag="tt1")
            rvn_cols = row_pool.tile([P, ms], F32, tag="rvncols")
            vts, tts = (vt0, vt1), (tt0, tt1)

            if kouter:
                proj_kouter(0, kt, wt_sb, bt_sb, txt_sb, tt0)
            else:
                for m in range(mh):
                    proj_group(0, m, kt, wt_sb, bt_sb, txt_sb, tt0)
            proj_group(1, 0, kt, wt_sb, bt_sb, txt_sb, tt1)
            ss_t0 = t_pre(tt0, 0)
            proj_group(1, 1, kt, wt_sb, bt_sb, txt_sb, tt1)
            rp_t0 = t_mid(ss_t0, 0)
            proj_group(1, 2, kt, wt_sb, bt_sb, txt_sb, tt1)
            t_fin(tt0, rp_t0, 0)
            proj_group(1, 3, kt, wt_sb, bt_sb, txt_sb, tt1)
            if emit_dmas is not None:
                emit_dmas()

            if kouter:
                proj_kouter(0, kv, wv_sb, bv_sb, vis_sb, vt0)
            else:
                for m in range(mh):
                    proj_group(0, m, kv, wv_sb, bv_sb, vis_sb, vt0)
            ss_t1 = t_pre(tt1, 1)
            ss_v0 = v_pre(vt0, 0)
            proj_group(1, 0, kv, wv_sb, bv_sb, vis_sb, vt1)
            proj_group(1, 1, kv, wv_sb, bv_sb, vis_sb, vt1)
            rp_t1 = t_mid(ss_t1, 1)
            proj_group(1, 2, kv, wv_sb, bv_sb, vis_sb, vt1)
            t_fin(tt1, rp_t1, 1)
            proj_group(1, 3, kv, wv_sb, bv_sb, vis_sb, vt1)

            # dots: first two i-blocks interleaved [i0jc0, i1jc0, i0jc1,
            # i1jc1] so the tt1-fold deadline lands two groups later; the V0
            # column chain threads through the slots between groups.
            pds0 = dots_pair(0, vts, tts)
            pn_v0 = v_ones(ss_v0)
            pds1 = dots_pair(1, vts, tts)
            rrow_v0 = v_recip(pn_v0, 0)
            v_cols(rrow_v0, rvn_cols, 0)
            ss_v1 = v_pre(vt1, 1)
            dots_epi(b, 0, pds0, rvn_cols)
            dots_epi(b, 1, pds1, rvn_cols)

            pds = dots_pair(2, vts, tts)
            pn_v1 = v_ones(ss_v1)
            dots_epi(b, 2, pds, rvn_cols)
            pds = dots_pair(3, vts, tts)
            rrow_v1 = v_recip(pn_v1, 1)
            dots_epi(b, 3, pds, rvn_cols)
            pds = dots_pair(4, vts, tts)
            v_cols(rrow_v1, rvn_cols, 1)
            dots_epi(b, 4, pds, rvn_cols, last=(b == bpc - 1 and 4 >= ms - 4))
            for i in range(5, ms):
                pds = dots_pair(i, vts, tts)
                dots_epi(b, i, pds, rvn_cols,
                         last=(b == bpc - 1 and i >= ms - 4))

        # ---------------- batch 0: chunk-paced head ----------------
        vis0_sb = xin_pool.tile([P, kv, s], CT, tag="vis")
        txt1_sb = xin_pool.tile([P, kt, s], CT, tag="txt")
        vis1_sb = xin_pool.tile([P, kv, s], CT, tag="vis")

        def emit_b0_dmas():
            # wv then vis b0 (two halves: the k-outer pass consumes at half
            # granularity); coarse DMAs so doorbell issue ops (~0.7us each)
            # don't pace the feed. Whole-tensor b1 loads queue up behind.
            nc.sync.dma_start(_w(wv_sb[:]), _w(wvp[:, :, :]))
            half = kv // 2
            nc.sync.dma_start(
                _w(vis0_sb[:, ds(0, half), :]),
                _w(visT[0, ds(0, half * P), :].rearrange("(k p) s -> p k s", p=P)),
            )
            nc.sync.dma_start(
                _w(vis0_sb[:, ds(half, half), :]),
                _w(visT[0, ds(half * P, half * P), :].rearrange(
                    "(k p) s -> p k s", p=P)),
            )
            nc.sync.dma_start(
                _w(txt1_sb[:]), _w(txtT[1, :, :].rearrange("(k p) s -> p k s", p=P))
            )
            nc.sync.dma_start(
                _w(vis1_sb[:]), _w(visT[1, :, :].rearrange("(k p) s -> p k s", p=P))
            )

        body(0, txt0_sb, vis0_sb, kouter=True, emit_dmas=emit_b0_dmas)
        body(1, txt1_sb, vis1_sb, kouter=False)

    nc.compile()
    return nc


_CACHE = {}


def _get_nc(dtype="bf16"):
    if dtype not in _CACHE:
        _CACHE[dtype] = build(dtype=dtype)
    return _CACHE[dtype]


def _prep_in_maps(visual_features, text_features, Wv, bv, Wt, bt, dtype="bf16"):
    import ml_dtypes

    f = np.float32
    ct = ml_dtypes.bfloat16 if dtype == "bf16" else f
    kv, kt, mh = VD // P, TD // P, H // P
    wvp = np.ascontiguousarray(
        np.asarray(Wv, dtype=f).T.reshape(kv, P, H).transpose(1, 0, 2)
    ).astype(ct)
    wtp = np.ascontiguousarray(
        np.asarray(Wt, dtype=f).T.reshape(kt, P, H).transpose(1, 0, 2)
    ).astype(ct)
    bvp = np.ascontiguousarray(np.asarray(bv, dtype=f).reshape(mh, P).T)
    btp = np.ascontiguousarray(np.asarray(bt, dtype=f).reshape(mh, P).T)
    ones = np.ones((P, P), dtype=np.float32).astype(ct)
    vis = np.asarray(visual_features, dtype=f)
    txt = np.asarray(text_features, dtype=f)
    in_maps = []
    for c in range(NCORES):
        sl = slice(c * BPC, (c + 1) * BPC)
        in_maps.append({
            "visT": np.ascontiguousarray(vis[sl].transpose(0, 2, 1)).astype(ct),
            "txtT": np.ascontiguousarray(txt[sl].transpose(0, 2, 1)).astype(ct),
            "wvp": wvp,
            "wtp": wtp,
            "bvp": bvp,
            "btp": btp,
            "ones": ones,
        })
    return in_maps


def _exec(nc, in_maps, trace, tmpdir):
    from concourse.bass_utils import run_bass_kernel_spmd

    res = run_bass_kernel_spmd(
        nc, in_maps, core_ids=list(range(NCORES)), trace=trace, tmpdir=tmpdir
    )
    outp = np.concatenate([res.results[c]["out"] for c in range(NCORES)], axis=0)
    return outp, res


def run(inputs, trace=False, tmpdir=None, dtype="bf16"):
    """Returns (full_output, BassKernelResults)."""
    nc = _get_nc(dtype)
    in_maps = _prep_in_maps(**inputs, dtype=dtype)
    return _exec(nc, in_maps, trace, tmpdir)


def kernel(**inputs) -> np.ndarray:
    """Full-input entry point. The first execution of a freshly-compiled
    NEFF occasionally returns a partially corrupted tile (observed ~2x in
    ~30 runs, always on the first exec), so execute twice and only accept
    a result confirmed by a second run."""
    nc = _get_nc("bf16")
    in_maps = _prep_in_maps(**inputs, dtype="bf16")
    prev = None
    for attempt in range(4):
        outp, _ = _exec(nc, in_maps, trace=False, tmpdir=None)
        if prev is not None:
            denom = np.linalg.norm(prev)
            if denom == 0.0 or np.linalg.norm(outp - prev) / denom < 1e-6:
                return outp
        prev = outp
    return prev
